# revision 1
# baseline (speedup 1.0000x reference)
"""Trainium2 Bass kernel for nn_CollaborativeExpertsWrapper.

Self-contained: shards batch B=128 across 8 NeuronCores (data-parallel
encoders), all-gathers [16, 2048] embeddings, each core redundantly computes
the masked ranking loss; host takes core 0's (loss, acc).
"""
import sys

sys.path.insert(0, "/opt/trn_rl_repo")

import math
from contextlib import ExitStack

import numpy as np

import concourse.bacc as bacc
import concourse.bass as bass
import concourse.mybir as mybir
import concourse.tile as tile
from concourse.alu_op_type import AluOpType
from concourse.masks import make_identity

F32 = mybir.dt.float32
F32R = mybir.dt.float32r
BF16 = mybir.dt.bfloat16
U8 = mybir.dt.uint8
AF = mybir.ActivationFunctionType
AX = mybir.AxisListType

N_CORES = 8
B = 128
BL = B // N_CORES  # 16 samples per core
T = 64
DIM = 512
HEADS = 4
HD = DIM // HEADS  # 128
MARGIN = 1.0
TOK = BL * T  # 1024 tokens per core per modality
O_T = 1024
ODIM = 512

_CACHE = {}


def _build():
    nc = bacc.Bacc("TRN2", target_bir_lowering=False, debug=False, num_devices=N_CORES)

    o_d = nc.dram_tensor("o", [BL, O_T, ODIM], F32, kind="ExternalInput").ap()
    rgb_d = nc.dram_tensor("rgb", [BL, T, 2048], F32, kind="ExternalInput").ap()
    aud_d = nc.dram_tensor("audio", [BL, T, 128], F32, kind="ExternalInput").ap()
    gm_d = nc.dram_tensor("group_mask", [B], U8, kind="ExternalInput").ap()

    wd = {}
    for m, dm in (("rgb", 2048), ("audio", 128)):
        for p in "qkv":
            wd[f"{m}_W{p}"] = nc.dram_tensor(f"{m}_W{p}", [dm, DIM], F32, kind="ExternalInput").ap()
            wd[f"{m}_b{p}"] = nc.dram_tensor(f"{m}_b{p}", [DIM], F32, kind="ExternalInput").ap()
        wd[f"{m}_Wo"] = nc.dram_tensor(f"{m}_Wo", [DIM, DIM], F32, kind="ExternalInput").ap()
        wd[f"{m}_bo"] = nc.dram_tensor(f"{m}_bo", [DIM], F32, kind="ExternalInput").ap()
        wd[f"{m}_W2"] = nc.dram_tensor(f"{m}_W2", [DIM, DIM], F32, kind="ExternalInput").ap()
        wd[f"{m}_b2"] = nc.dram_tensor(f"{m}_b2", [DIM], F32, kind="ExternalInput").ap()
    wd["expand_W"] = nc.dram_tensor("expand_W", [DIM, 2 * DIM], F32, kind="ExternalInput").ap()
    wd["expand_b"] = nc.dram_tensor("expand_b", [2 * DIM], F32, kind="ExternalInput").ap()

    out_d = nc.dram_tensor("out", [1, 2], F32, kind="ExternalOutput").ap()

    import os
    stage = os.environ.get("KSTAGE", "full")
    dbg_d = None
    if stage != "full":
        dbg_d = nc.dram_tensor("dbg", [B, 4 * DIM], F32, kind="ExternalOutput").ap()

    with tile.TileContext(nc) as tc:
        _emit(nc, tc, o_d, rgb_d, aud_d, gm_d, wd, out_d, stage, dbg_d)

    nc.compile()
    return nc


def _emit(nc, tc, o_d, rgb_d, aud_d, gm_d, wd, out_d, stage="full", dbg_d=None):
    stk = ExitStack()
    with stk:
        const = stk.enter_context(tc.tile_pool(name="const", bufs=1))
        persist = stk.enter_context(tc.tile_pool(name="persist", bufs=1))
        ps = stk.enter_context(tc.tile_pool(name="psum", bufs=7, space="PSUM"))
        dram = stk.enter_context(tc.tile_pool(name="dram", bufs=1, space="DRAM"))

        def pst(shape, tag="ps", bufs=None):
            return ps.tile(shape, F32, tag=tag, bufs=bufs, name=tag)

        # ---------------- constants ----------------
        ident = const.tile([128, 128], F32, tag="ident")
        make_identity(nc, ident)
        ones_col_f32 = const.tile([128, 1], F32, tag="ones_col_f32")
        nc.vector.memset(ones_col_f32[:], 1.0)
        ones64_s = const.tile([128, 128], F32, tag="ones64_s")
        nc.vector.memset(ones64_s[:], 0.0)
        nc.vector.memset(ones64_s[0:64, 0:64], 1.0)
        nc.vector.memset(ones64_s[64:128, 64:128], 1.0)
        ones64_r = const.tile([128, 128], F32R, tag="ones64")
        nc.vector.tensor_copy(ones64_r[:], ones64_s[:])
        ones_row_f32 = const.tile([1, 128], F32, tag="ones_row_f32")
        nc.vector.memset(ones_row_f32[:], 1.0)
        ones128 = const.tile([128, 128], F32, tag="ones128")
        nc.vector.memset(ones128[:], 1.0)
        ones_row_r = const.tile([1, 128], F32R, tag="ones_row_r")
        nc.vector.tensor_copy(ones_row_r[:], ones_row_f32[:])
        sel16_s = const.tile([128, BL, BL], F32, tag="sel16_s")
        nc.vector.memset(sel16_s[:], 0.0)
        for b in range(BL):
            nc.vector.memset(sel16_s[:, b, b : b + 1], 1.0)
        sel16 = const.tile([128, BL, BL], BF16, tag="sel16")
        nc.vector.tensor_copy(sel16[:], sel16_s[:])
        ones64_bf = const.tile([64, 64], BF16, tag="ones64_bf")
        nc.vector.tensor_copy(ones64_bf[:], ones64_s[0:64, 0:64])

        g_row_u8 = const.tile([1, B], U8, tag="g_row_u8")
        nc.sync.dma_start(g_row_u8[:], gm_d[None, :])
        g_row = const.tile([1, B], F32, tag="g_row")
        nc.vector.tensor_copy(g_row[:], g_row_u8[:])
        g_col_u8 = const.tile([B, 1], U8, tag="g_col_u8")
        nc.sync.dma_start(g_col_u8[:], gm_d[:, None])
        g_col = const.tile([B, 1], F32, tag="g_col")
        nc.vector.tensor_copy(g_col[:], g_col_u8[:])
        gneg_row = const.tile([1, B], F32, tag="gneg_row")
        nc.vector.tensor_scalar(gneg_row[:], g_row[:], 1e30, -1e30, AluOpType.mult, AluOpType.add)

        feat_sb = persist.tile([BL, 2 * DIM], F32, tag="feat")
        oo_sb = persist.tile([BL, 2 * DIM], F32, tag="oo")

        # o tiles pool opened early so its space never WAR-blocks on encoder pools
        o_pool = stk.enter_context(tc.tile_pool(name="o_pool", bufs=3))

        # ---------------- rgb encoder (its weight DMAs queue ahead of the o stream) ------
        _encoder(nc, tc, pst, persist, const, "rgb", 2048, rgb_d, wd, feat_sb, 0,
                 ident, ones_row_r, ones64_bf)

        # expand weights loaded early (small; unblocks the expand chain)
        expw_pool = stk.enter_context(tc.tile_pool(name="expw", bufs=1))
        expw = expw_pool.tile([128, 4, 2 * DIM], F32R, tag="expw")
        nc.gpsimd.dma_start(expw[:], wd["expand_W"].rearrange("(c p) d -> p c d", p=128))
        expb = expw_pool.tile([1, 2 * DIM], F32R, tag="expb")
        nc.gpsimd.dma_start(expb[:], wd["expand_b"][None, :])

        # ---------------- audio encoder ----------------
        _encoder(nc, tc, pst, persist, const, "audio", 128, aud_d, wd,
                 feat_sb, DIM, ident, ones_row_r, ones64_bf)

        if stage == "enc":
            nc.sync.dma_start(dbg_d[0:BL, 0 : 2 * DIM], feat_sb[:])
            return

        # ---------------- o-mean (bf16 stream, overlaps encoder tail) ----------------
        om_ps = pst([BL, ODIM], tag="ps_om", bufs=1)
        o_view = o_d.rearrange("b (n p) d -> b p n d", p=128)
        for b in range(BL):
            o_sb = o_pool.tile([128, O_T // 128, ODIM], BF16, tag="o_tile")
            nc.gpsimd.dma_start(o_sb[:], o_view[b])
            for j in range(O_T // 128):
                nc.tensor.matmul(
                    om_ps[:],
                    sel16[:, b, :],
                    o_sb[:, j, :],
                    start=(b == 0 and j == 0),
                    stop=(b == BL - 1 and j == O_T // 128 - 1),
                )

        om_sb = persist.tile([BL, ODIM], F32, tag="om")
        nc.scalar.activation(om_sb[:], om_ps[:], AF.Copy, scale=1.0 / O_T)
        omT = persist.tile([128, 4, BL], F32R, tag="omT")
        for c in range(4):
            tp = pst([128, BL])
            nc.tensor.transpose(tp[:], om_sb[:, 128 * c : 128 * (c + 1)], ident[:BL, :BL])
            nc.scalar.copy(omT[:, c, :], tp[:])

        # ---------------- expand + normalize -> oo ----------------
        if True:
            oo_ps = []
            for half in range(2):
                pp = pst([BL, DIM])
                for c in range(4):
                    nc.tensor.matmul(pp[:], omT[:, c, :], expw[:, c, 512 * half : 512 * (half + 1)],
                                     start=(c == 0), stop=False)
                nc.tensor.matmul(pp[:], ones_row_r[:, :BL], expb[:, 512 * half : 512 * (half + 1)],
                                 start=False, stop=True)
                oo_ps.append(pp)
            sq_junk = persist.tile([BL, DIM], F32, tag="sq_junk")
            ss = [persist.tile([BL, 1], F32, tag=f"ss{i}", name=f"ss{i}") for i in range(2)]
            for half in range(2):
                nc.scalar.activation(sq_junk[:], oo_ps[half][:], AF.Square, accum_out=ss[half][:])
            nrm = persist.tile([BL, 1], F32, tag="nrm")
            nc.vector.tensor_tensor(nrm[:], ss[0][:], ss[1][:], AluOpType.add)
            nc.scalar.sqrt(nrm[:], nrm[:])
            nc.vector.tensor_scalar_max(nrm[:], nrm[:], 1e-12)
            rnrm = persist.tile([BL, 1], F32, tag="rnrm")
            nc.vector.reciprocal(rnrm[:], nrm[:])
            for half in range(2):
                nc.vector.tensor_scalar_mul(oo_sb[:, 512 * half : 512 * (half + 1)],
                                            oo_ps[half][:], rnrm[:])


        if stage == "oenc":
            nc.sync.dma_start(dbg_d[0:BL, 0 : 2 * DIM], feat_sb[:])
            nc.sync.dma_start(dbg_d[0:BL, 2 * DIM :], oo_sb[:])
            return

        # ---------------- AllGather ----------------
        ag_in = dram.tile([BL, 4 * DIM], F32)
        ag_out = dram.tile([B, 4 * DIM], F32)
        nc.sync.dma_start(ag_in[:, : 2 * DIM], feat_sb[:])
        nc.sync.dma_start(ag_in[:, 2 * DIM :], oo_sb[:])
        import os
        if os.environ.get("KTIME"):
            # collective-free stand-in for TimelineSim (cost model can't model
            # collectives); timing-equivalent except the ~15us AllGather.
            nc.sync.dma_start(ag_out[0:BL, :], ag_in[:])
        else:
            nc.gpsimd.collective_compute(
                "AllGather",
                AluOpType.bypass,
                replica_groups=[list(range(N_CORES))],
                ins=[ag_in.opt()],
                outs=[ag_out.opt()],
            )

        # ---------------- ranking ----------------
        with tc.tile_pool(name="rank", bufs=1) as rank_pool:
            emb = rank_pool.tile([B, 4 * DIM], F32, tag="emb")
            nc.sync.dma_start(emb[:], ag_out[:])

            if stage == "ag":
                nc.sync.dma_start(dbg_d[:], emb[:])
                return

            # transpose emb -> embT [128, 16, 128]; chunks 0..7 featT, 8..15 ooT
            embT = rank_pool.tile([128, 16, 128], F32, tag="embT")
            for grp4 in range(4):
                tp = pst([128, 512])
                for j in range(4):
                    c = 4 * grp4 + j
                    nc.tensor.transpose(tp[:, 128 * j : 128 * (j + 1)],
                                        emb[:, 128 * c : 128 * (c + 1)], ident[:])
                nc.scalar.copy(embT[:, 4 * grp4 : 4 * grp4 + 4, :],
                               tp[:].rearrange("p (j c) -> p j c", j=4))

            G_ps = pst([B, B])
            for c in range(8):
                nc.tensor.matmul(G_ps[:], embT[:, 8 + c, :], embT[:, c, :],
                                 start=(c == 0), stop=(c == 7))
            G_sb = rank_pool.tile([B, B], F32, tag="G_sb")
            nc.scalar.copy(G_sb[:], G_ps[:])

            if stage == "rank1":
                nc.sync.dma_start(dbg_d[:, 0:B], G_sb[:])
                return

            junk = rank_pool.tile([B, B], F32, tag="junk")
            diag = rank_pool.tile([B, 1], F32, tag="diag")
            nc.vector.tensor_tensor(junk[:], G_sb[:], ident[:], AluOpType.mult)
            nc.vector.reduce_sum(diag[:], junk[:], axis=AX.X)
            mdiag = rank_pool.tile([B, 1], F32, tag="mdiag")
            nc.vector.tensor_scalar(mdiag[:], diag[:], -1.0, MARGIN,
                                    AluOpType.mult, AluOpType.add)

            Gt_ps = pst([B, B])
            nc.tensor.transpose(Gt_ps[:], G_sb[:], ident[:])
            Gt_sb = rank_pool.tile([B, B], F32, tag="Gt_sb")
            nc.scalar.copy(Gt_sb[:], Gt_ps[:])

            if stage == "rank1b":
                nc.sync.dma_start(dbg_d[:, 0:B], Gt_sb[:])
                nc.sync.dma_start(dbg_d[:, B : B + 1], diag[:])
                return

            # broadcast g along partitions: gb[m, n] = g[n], via colsums of a
            # zero-padded one-row matrix (K=1 matmuls are avoided).
            g_pad = rank_pool.tile([B, B], F32, tag="g_pad")
            nc.vector.memset(g_pad[:], 0.0)
            nc.vector.tensor_copy(g_pad[0:1, :], g_row[:])
            gb_ps = pst([B, B])
            nc.tensor.matmul(gb_ps[:], ones128[:], g_pad[:], start=True, stop=True)
            gneg_sb = rank_pool.tile([B, B], F32, tag="gneg_sb")
            nc.vector.tensor_scalar(gneg_sb[:], gb_ps[:], 1e30, -1e30,
                                    AluOpType.mult, AluOpType.add)

            stack = rank_pool.tile([B, 6], F32, tag="stack")
            Gm = rank_pool.tile([B, B], F32, tag="Gm")
            rmax = rank_pool.tile([B, 1], F32, tag="rmax")
            top = rank_pool.tile([B, 1], F32, tag="top")
            w = rank_pool.tile([B, 1], F32, tag="w")
            sel = rank_pool.tile([B, 1], F32, tag="sel")
            eq = rank_pool.tile([B, 1], F32, tag="eq")
            colv = rank_pool.tile([B, 1], F32, tag="colv")

            for di, Gsrc in enumerate((G_sb, Gt_sb)):
                T_sb = rank_pool.tile([B, B], F32, tag=f"T{di}")
                nc.scalar.activation(T_sb[:], Gsrc[:], AF.Relu, bias=mdiag[:])
                nc.vector.tensor_tensor(junk[:], T_sb[:], gb_ps[:], AluOpType.mult)
                nc.vector.reduce_sum(w[:], junk[:], axis=AX.X)
                nc.vector.tensor_tensor(stack[:, di : di + 1], w[:], g_col[:], AluOpType.mult)
                nc.vector.tensor_tensor(Gm[:], Gsrc[:], gneg_sb[:], AluOpType.add)
                nc.vector.reduce_max(rmax[:], Gm[:], axis=AX.X)
                nc.vector.tensor_tensor(top[:], diag[:], rmax[:], AluOpType.is_ge)
                nc.vector.tensor_tensor(junk[:], Gsrc[:], gb_ps[:], AluOpType.mult)
                nc.vector.reduce_sum(sel[:], junk[:], axis=AX.X)
                nc.vector.tensor_tensor(sel[:], sel[:], g_col[:], AluOpType.mult)
                nc.vector.tensor_scalar(eq[:], sel[:], 0.0, None, AluOpType.is_equal)
                nc.vector.tensor_scalar(colv[:], eq[:], -1.0, 1.0,
                                        AluOpType.mult, AluOpType.add)
                nc.vector.tensor_copy(stack[:, 4 + di : 5 + di], colv[:])
                nc.vector.tensor_tensor(stack[:, 2 + di : 3 + di], colv[:], top[:],
                                        AluOpType.mult)

            if stage == "rank2":
                nc.sync.dma_start(dbg_d[:, 0:6], stack[:])
                nc.sync.dma_start(dbg_d[:, 8:136], Gt_sb[:])
                return

            S_ps = pst([1, 6])
            nc.tensor.matmul(S_ps[:], ones_col_f32[:], stack[:], start=True, stop=True)
            S_sb = rank_pool.tile([1, 6], F32, tag="S_sb")
            nc.vector.tensor_copy(S_sb[:], S_ps[:])

            if stage == "rank3":
                nc.sync.dma_start(dbg_d[0:1, 0:6], S_sb[:])
                return

            sg = rank_pool.tile([1, 1], F32, tag="sg")
            nc.vector.reduce_sum(sg[:], g_row[:], axis=AX.X)

            def sc(tag):
                return rank_pool.tile([1, 1], F32, tag=tag, name=tag)

            t_ls = sc("t_ls")
            nc.vector.tensor_tensor(t_ls[:], S_sb[:, 0:1], S_sb[:, 1:2], AluOpType.add)
            num = sc("num")
            nc.vector.tensor_scalar_mul(num[:], sg[:], -2.0 * MARGIN)
            nc.vector.tensor_tensor(num[:], num[:], t_ls[:], AluOpType.add)
            d1 = sc("d1")
            nc.vector.tensor_scalar(d1[:], sg[:], -1.0, 1.0, AluOpType.add, AluOpType.max)
            ind = sc("ind")
            nc.vector.tensor_scalar(ind[:], sg[:], -1.0, 0.0, AluOpType.add, AluOpType.max)
            nc.vector.tensor_scalar_min(ind[:], ind[:], 1.0)
            nv = sc("nv")
            nc.vector.tensor_tensor(nv[:], ind[:], sg[:], AluOpType.mult)
            d2 = sc("d2")
            nc.vector.tensor_scalar_max(d2[:], nv[:], 1.0)
            r1 = sc("r1")
            nc.vector.reciprocal(r1[:], d1[:])
            r2 = sc("r2")
            nc.vector.reciprocal(r2[:], d2[:])
            out_sb = rank_pool.tile([1, 2], F32, tag="out_sb")
            nc.vector.tensor_tensor(num[:], num[:], r1[:], AluOpType.mult)
            nc.vector.tensor_tensor(out_sb[:, 0:1], num[:], r2[:], AluOpType.mult)

            acc_h = []
            for di in range(2):
                nvx = sc(f"nvx{di}")
                nc.vector.tensor_scalar_max(nvx[:], S_sb[:, 4 + di : 5 + di], 1.0)
                rx = sc(f"rx{di}")
                nc.vector.reciprocal(rx[:], nvx[:])
                ax = sc(f"ax{di}")
                nc.vector.tensor_tensor(ax[:], S_sb[:, 2 + di : 3 + di], rx[:], AluOpType.mult)
                acc_h.append(ax)
            asum = sc("asum")
            nc.vector.tensor_tensor(asum[:], acc_h[0][:], acc_h[1][:], AluOpType.add)
            nc.vector.tensor_scalar_mul(out_sb[:, 1:2], asum[:], 0.5)

            nc.sync.dma_start(out_d[:], out_sb[:])


def _encoder(nc, tc, pst, persist, const, mod, dm, x_d, wd, feat_sb, feat_off,
             ident, ones_row_r, ones64_bf):
    """Self-attention pooled encoder; writes feat_sb[:, feat_off:feat_off+512]."""
    K = dm // 128
    n_tt = TOK // 128  # 8

    enc_pool_cm = tc.tile_pool(name=f"enc_{mod}", bufs=1)
    enc = enc_pool_cm.__enter__()
    qT = enc.tile([128, HEADS, TOK], BF16, tag="qT")
    kT = enc.tile([128, HEADS, TOK], BF16, tag="kT")
    v_sb = enc.tile([128, n_tt, DIM], BF16, tag="v_sb")
    poolT = enc.tile([128, HEADS, BL], F32R, tag="poolT")

    with ExitStack() as estk:
        xT_pool = estk.enter_context(tc.tile_pool(name=f"xT_{mod}", bufs=1))
        xT = xT_pool.tile([128, K, TOK], F32R, tag="xT")
        flat = x_d.rearrange("b t d -> (b t) d")
        with tc.tile_pool(name=f"xload_{mod}", bufs=2) as xload:
            if dm == 128:
                x_nat = xload.tile([128, n_tt, 128], F32, tag="x_nat_a")
                nc.sync.dma_start(x_nat[:], flat.rearrange("(n p) d -> p n d", p=128))
                for tt in range(n_tt):
                    tp = pst([128, 512])
                    nc.tensor.transpose(tp[:, :128], x_nat[:, tt, :], ident[:])
                    nc.scalar.copy(xT[:, 0, 128 * tt : 128 * (tt + 1)], tp[:, :128])
            else:
                for tt in range(n_tt):
                    x_nat = xload.tile([128, dm], F32, tag="x_nat")
                    nc.sync.dma_start(x_nat[:], flat[128 * tt : 128 * (tt + 1), :])
                    for kc4 in range(K // 4):
                        tp = pst([128, 512])
                        for j in range(4):
                            kc = 4 * kc4 + j
                            nc.tensor.transpose(tp[:, 128 * j : 128 * (j + 1)],
                                                x_nat[:, 128 * kc : 128 * (kc + 1)], ident[:])
                        nc.scalar.copy(xT[:, 4 * kc4 : 4 * kc4 + 4, 128 * tt : 128 * (tt + 1)],
                                       tp[:].rearrange("p (j c) -> p j c", j=4))

        # v: lhsT = xT token-tile (stationary), rhs = Wv k-rows (moving)
        with tc.tile_pool(name=f"wv_{mod}", bufs=1) as wv_pool:
            wv = wv_pool.tile([128, K, DIM], F32R, tag="wv")
            nc.gpsimd.dma_start(wv[:], wd[f"{mod}_Wv"].rearrange("(kc p) d -> p kc d", p=128))
            bv = wv_pool.tile([1, DIM], F32R, tag="bv")
            nc.gpsimd.dma_start(bv[:], wd[f"{mod}_bv"][None, :])
            for tt in range(n_tt):
                pv = pst([128, DIM])
                for kc in range(K):
                    nc.tensor.matmul(pv[:], xT[:, kc, 128 * tt : 128 * (tt + 1)], wv[:, kc, :],
                                     start=(kc == 0), stop=False)
                nc.tensor.matmul(pv[:], ones_row_r[:], bv[:], start=False, stop=True)
                nc.vector.tensor_copy(v_sb[:, tt, :], pv[:])

        # q, k: lhsT = W column-block (stationary), rhs = xT (moving) -> [d, tok]
        bq_sb = const.tile([128, HEADS], F32, tag=f"bq_{mod}")
        nc.sync.dma_start(bq_sb[:], wd[f"{mod}_bq"].rearrange("(o p) -> p o", p=128))
        bk_sb = const.tile([128, HEADS], F32, tag=f"bk_{mod}")
        nc.sync.dma_start(bk_sb[:], wd[f"{mod}_bk"].rearrange("(o p) -> p o", p=128))
        with tc.tile_pool(name=f"wcol_{mod}", bufs=2) as wcol_pool:
            for pname, outT, b_sb in (("q", qT, bq_sb), ("k", kT, bk_sb)):
                w_d = wd[f"{mod}_W{pname}"].rearrange("(kc p) d -> p kc d", p=128)
                if K == 1:
                    wfull = wcol_pool.tile([128, DIM], F32R, tag="wfull", name="wfull")
                    nc.gpsimd.dma_start(wfull[:], w_d[:, 0, :])
                for dt_ in range(HEADS):
                    if K == 1:
                        wcol = wfull[:, None, 128 * dt_ : 128 * (dt_ + 1)]
                    else:
                        wcol = wcol_pool.tile([128, K, 128], F32R, tag="wcol",
                                              name="wcol")
                        nc.gpsimd.dma_start(wcol[:],
                                            w_d[:, :, 128 * dt_ : 128 * (dt_ + 1)])
                    for blk in range(TOK // 512):
                        pq = pst([128, 512])
                        for kc in range(K):
                            nc.tensor.matmul(pq[:], wcol[:, kc, :],
                                             xT[:, kc, 512 * blk : 512 * (blk + 1)],
                                             start=(kc == 0), stop=(kc == K - 1))
                        nc.scalar.activation(outT[:, dt_, 512 * blk : 512 * (blk + 1)], pq[:],
                                             AF.Identity, bias=b_sb[:, dt_ : dt_ + 1])

    # attention, grp-outer: reshuffle 8 samples of v to base partition 0 via
    # SBUF->SBUF DMA (engines cannot shift partitions), then per-head flow.
    scale = 1.0 / math.sqrt(HD)
    with ExitStack() as lstk:
        late = lstk.enter_context(tc.tile_pool(name=f"late_{mod}", bufs=1))
        avT = late.tile([128, HEADS, TOK], F32R, tag="avT")
        wo_pool = lstk.enter_context(tc.tile_pool(name=f"wo_{mod}", bufs=1))
        wo = wo_pool.tile([128, HEADS, DIM], F32R, tag="wo")
        nc.gpsimd.dma_start(wo[:], wd[f"{mod}_Wo"].rearrange("(h p) d -> p h d", p=128))
        w2 = wo_pool.tile([128, HEADS, DIM], F32R, tag="w2")
        nc.gpsimd.dma_start(w2[:], wd[f"{mod}_W2"].rearrange("(c p) d -> p c d", p=128))
        b2 = wo_pool.tile([1, DIM], F32R, tag="b2")
        nc.gpsimd.dma_start(b2[:], wd[f"{mod}_b2"][None, :])
        bo_sb = const.tile([128, HEADS], F32, tag=f"bo_{mod}")
        nc.sync.dma_start(bo_sb[:], wd[f"{mod}_bo"].rearrange("(o p) -> p o", p=128))
        ap = lstk.enter_context(tc.tile_pool(name=f"attn_{mod}", bufs=3))
        for grp in range(BL // 8):
            v8 = ap.tile([64, 8, DIM], BF16, tag="v8")
            v8v = v8[:].rearrange("p (ul half) d -> p ul half d", half=2)
            nc.sync.dma_start(v8v[:, :, 0, :], v_sb[0:64, 4 * grp : 4 * grp + 4, :])
            nc.sync.dma_start(v8v[:, :, 1, :], v_sb[64:128, 4 * grp : 4 * grp + 4, :])
            for h in range(HEADS):
                sT8 = pst([64, 512])
                for i in range(8):
                    b = 8 * grp + i
                    nc.tensor.matmul(sT8[:, 64 * i : 64 * (i + 1)],
                                     kT[:, h, 64 * b : 64 * (b + 1)],
                                     qT[:, h, 64 * b : 64 * (b + 1)],
                                     start=True, stop=True)
                exps = ap.tile([64, 512], BF16, tag="exps")
                nc.scalar.activation(exps[:], sT8[:], AF.Exp, scale=scale)
                rs = pst([64, 512])
                nc.tensor.matmul(rs[:], ones64_bf[:], exps[:],
                                 start=True, stop=True)
                rrs = ap.tile([64, 512], F32, tag="rrs")
                nc.vector.reciprocal(rrs[:], rs[:])
                aT8 = ap.tile([64, 512], BF16, tag="aT8")
                nc.vector.tensor_tensor(aT8[:], exps[:], rrs[:], AluOpType.mult)
                avp = pst([128, 512])
                for i in range(8):
                    nc.tensor.matmul(avp[:, 64 * i : 64 * (i + 1)],
                                     v8[:, i, 128 * h : 128 * (h + 1)],
                                     aT8[:, 64 * i : 64 * (i + 1)],
                                     start=True, stop=True)
                nc.vector.tensor_copy(avT[:, h, 512 * grp : 512 * (grp + 1)], avp[:])

        # out-proj (transposed) + time pooling + W2
        _proj_w2(nc, tc, pst, wo_pool, mod, feat_sb, feat_off, avT, poolT,
                 ones_row_r, wo, w2, b2, bo_sb)

    enc_pool_cm.__exit__(None, None, None)


def _proj_w2(nc, tc, pst, wo_pool, mod, feat_sb, feat_off, avT, poolT,
             ones_row_r, wo, w2, b2, bo_sb):
    if True:
        red = wo_pool.tile([128, 8], F32, tag="red")
        for dt_ in range(HEADS):
            for blk in range(TOK // 512):
                pp = pst([128, 512])
                for h in range(HEADS):
                    nc.tensor.matmul(pp[:], wo[:, h, 128 * dt_ : 128 * (dt_ + 1)],
                                     avT[:, h, 512 * blk : 512 * (blk + 1)],
                                     start=(h == 0), stop=(h == HEADS - 1))
                nc.vector.reduce_sum(red[:], pp[:].rearrange("p (s t) -> p s t", t=T),
                                     axis=AX.X)
                nc.vector.tensor_scalar(poolT[:, dt_, 8 * blk : 8 * blk + 8], red[:],
                                        1.0 / T, bo_sb[:, dt_ : dt_ + 1],
                                        AluOpType.mult, AluOpType.add)

        pf = pst([BL, DIM])
        for c in range(HEADS):
            nc.tensor.matmul(pf[:], poolT[:, c, :], w2[:, c, :], start=(c == 0), stop=False)
        nc.tensor.matmul(pf[:], ones_row_r[:, :BL], b2[:], start=False, stop=True)
        nc.scalar.copy(feat_sb[:, feat_off : feat_off + DIM], pf[:])


def kernel(**inputs):
    if "runner" not in _CACHE:
        _CACHE["runner"] = _make_runner()
    return _CACHE["runner"](inputs)


def _make_runner():
    nc = _build()
    import jax
    from jax.sharding import Mesh, PartitionSpec
    from jax.experimental.shard_map import shard_map
    from concourse import bass2jax

    bass2jax.install_neuronx_cc_hook()

    partition_name = nc.partition_id_tensor.name if nc.partition_id_tensor else None
    in_names, out_names, out_avals, zero_outs = [], [], [], []
    for alloc in nc.m.functions[0].allocations:
        if not isinstance(alloc, mybir.MemoryLocationSet):
            continue
        name = alloc.memorylocations[0].name
        if alloc.kind == "ExternalInput":
            if name != partition_name:
                in_names.append(name)
        elif alloc.kind == "ExternalOutput":
            out_names.append(name)
            shape = tuple(alloc.tensor_shape)
            dtype = mybir.dt.np(alloc.dtype)
            out_avals.append(jax.core.ShapedArray(shape, dtype))
            zero_outs.append(np.zeros(shape, dtype))
    n_params = len(in_names)
    all_in_names = list(in_names) + list(out_names)
    if partition_name is not None:
        all_in_names.append(partition_name)

    def _body(*args):
        operands = list(args)
        if partition_name is not None:
            operands.append(bass2jax.partition_id_tensor())
        outs = bass2jax._bass_exec_p.bind(
            *operands,
            out_avals=tuple(out_avals),
            in_names=tuple(all_in_names),
            out_names=tuple(out_names),
            lowering_input_output_aliases=(),
            sim_require_finite=True,
            sim_require_nnan=True,
            nc=nc,
        )
        return tuple(outs)

    devices = jax.devices()[:N_CORES]
    mesh = Mesh(np.asarray(devices), ("core",))
    in_specs = (PartitionSpec("core"),) * (n_params + len(out_names))
    out_specs = (PartitionSpec("core"),) * len(out_names)
    sharded = jax.jit(
        shard_map(_body, mesh=mesh, in_specs=in_specs, out_specs=out_specs,
                  check_rep=False),
        keep_unused=True,
    )

    out_idx = out_names.index("out")

    def run(inputs):
        per_core = _shard_inputs(inputs)
        concat_in = [
            np.concatenate([per_core[c][name] for c in range(N_CORES)], axis=0)
            for name in in_names
        ]
        concat_zeros = [
            np.zeros((N_CORES * z.shape[0], *z.shape[1:]), z.dtype) for z in zero_outs
        ]
        out_arrs = sharded(*concat_in, *concat_zeros)
        run.last_outputs = {n: np.asarray(out_arrs[i]) for i, n in enumerate(out_names)}
        out = run.last_outputs["out"]  # [8, 2]
        return np.float32(out[0, 0]), np.float32(out[0, 1])

    run.sharded = sharded
    run.in_names = in_names
    run.zero_outs = zero_outs
    run.nc = nc
    return run


def _shard_inputs(inputs):
    per_core = []
    gm = np.ascontiguousarray(np.asarray(inputs["group_mask"]).astype(np.uint8))
    shared = {}
    for k, v in inputs.items():
        if k not in ("o", "rgb", "audio", "group_mask"):
            shared[k] = np.ascontiguousarray(np.asarray(v, dtype=np.float32))
    o = np.asarray(inputs["o"], dtype=np.float32)
    rgb = np.asarray(inputs["rgb"], dtype=np.float32)
    audio = np.asarray(inputs["audio"], dtype=np.float32)
    for c in range(N_CORES):
        sl = slice(BL * c, BL * (c + 1))
        m = {
            "o": np.ascontiguousarray(o[sl]),
            "rgb": np.ascontiguousarray(rgb[sl]),
            "audio": np.ascontiguousarray(audio[sl]),
            "group_mask": gm,
        }
        m.update(shared)
        per_core.append(m)
    return per_core



# revision 15
# speedup vs baseline: 187.2742x; 187.2742x over previous
"""Trainium2 Bass kernel for nn_CollaborativeExpertsWrapper.

Self-contained: shards batch B=128 across 8 NeuronCores (data-parallel
encoders), all-gathers [16, 2048] embeddings, each core redundantly computes
the masked ranking loss; host takes core 0's (loss, acc).

v2: o-mean stream on a dedicated HWDGE queue (f32r, no cast) with its PE
matmuls interleaved ("pumped") through the encoder compute; shared-scratchpad
AllGather output; all other DMAs on the scalar/gpsimd queues.
"""
import sys

sys.path.insert(0, "/opt/trn_rl_repo")

import math
import os
from contextlib import ExitStack

import numpy as np

import concourse.bacc as bacc
import concourse.bass as bass
import concourse.mybir as mybir
import concourse.tile as tile
from concourse.alu_op_type import AluOpType
from concourse.masks import make_identity

F32 = mybir.dt.float32
F32R = mybir.dt.float32r
BF16 = mybir.dt.bfloat16
U8 = mybir.dt.uint8
AF = mybir.ActivationFunctionType
AX = mybir.AxisListType

N_CORES = 8
B = 128
BL = B // N_CORES  # 16 samples per core
T = 64
DIM = 512
HEADS = 4
HD = DIM // HEADS  # 128
MARGIN = 1.0
TOK = BL * T  # 1024 tokens per core per modality
O_T = 1024
ODIM = 512
O_BUFS = 4  # SBUF staging tiles for the o stream (1MB each, half a sample)

_CACHE = {}


def _build():
    nc = bacc.Bacc("TRN2", target_bir_lowering=False, debug=False, num_devices=N_CORES)

    # o and the weights are declared f32r (same bytes as f32) so plain HWDGE
    # loads feed the PE's full-rate f32r path with no cast DMA.
    o_d = nc.dram_tensor("o", [BL, O_T, ODIM], F32R, kind="ExternalInput").ap()
    rgb_d = nc.dram_tensor("rgb", [BL, T, 2048], F32, kind="ExternalInput").ap()
    aud_d = nc.dram_tensor("audio", [BL, T, 128], F32, kind="ExternalInput").ap()
    gm_d = nc.dram_tensor("group_mask", [B], U8, kind="ExternalInput").ap()

    wd = {}
    for m, dm in (("rgb", 2048), ("audio", 128)):
        for p in "qkv":
            wd[f"{m}_W{p}"] = nc.dram_tensor(f"{m}_W{p}", [dm, DIM], F32R, kind="ExternalInput").ap()
            wd[f"{m}_b{p}"] = nc.dram_tensor(f"{m}_b{p}", [DIM], F32, kind="ExternalInput").ap()
        wd[f"{m}_Wo"] = nc.dram_tensor(f"{m}_Wo", [DIM, DIM], F32R, kind="ExternalInput").ap()
        wd[f"{m}_bo"] = nc.dram_tensor(f"{m}_bo", [DIM], F32, kind="ExternalInput").ap()
        wd[f"{m}_W2"] = nc.dram_tensor(f"{m}_W2", [DIM, DIM], F32R, kind="ExternalInput").ap()
        wd[f"{m}_b2"] = nc.dram_tensor(f"{m}_b2", [DIM], F32, kind="ExternalInput").ap()
    wd["expand_W"] = nc.dram_tensor("expand_W", [DIM, 2 * DIM], F32R, kind="ExternalInput").ap()
    wd["expand_b"] = nc.dram_tensor("expand_b", [2 * DIM], F32, kind="ExternalInput").ap()

    out_d = nc.dram_tensor("out", [1, 2], F32, kind="ExternalOutput").ap()

    stage = os.environ.get("KSTAGE", "full")
    dbg_d = None
    if stage != "full":
        dbg_d = nc.dram_tensor("dbg", [B, 4 * DIM], F32, kind="ExternalOutput").ap()

    with tile.TileContext(nc) as tc:
        _emit(nc, tc, o_d, rgb_d, aud_d, gm_d, wd, out_d, stage, dbg_d)

    nc.compile()
    return nc


def _emit(nc, tc, o_d, rgb_d, aud_d, gm_d, wd, out_d, stage="full", dbg_d=None):
    stk = ExitStack()
    with stk:
        const = stk.enter_context(tc.tile_pool(name="const", bufs=1))
        persist = stk.enter_context(tc.tile_pool(name="persist", bufs=1))
        ps = stk.enter_context(tc.tile_pool(name="psum", bufs=6, space="PSUM"))
        dram = stk.enter_context(tc.tile_pool(name="dram", bufs=1, space="DRAM"))

        def pst(shape, tag="ps", bufs=None):
            return ps.tile(shape, F32, tag=tag, bufs=bufs, name=tag)

        # ---------------- constants ----------------
        ident = const.tile([128, 128], F32, tag="ident")
        make_identity(nc, ident)
        ones_col_f32 = const.tile([128, 1], F32, tag="ones_col_f32")
        nc.vector.memset(ones_col_f32[:], 1.0)
        ones64_s = const.tile([128, 128], F32, tag="ones64_s")
        nc.vector.memset(ones64_s[:], 0.0)
        nc.vector.memset(ones64_s[0:64, 0:64], 1.0)
        nc.vector.memset(ones64_s[64:128, 64:128], 1.0)
        ones_row_f32 = const.tile([1, 128], F32, tag="ones_row_f32")
        nc.vector.memset(ones_row_f32[:], 1.0)
        ones128 = const.tile([128, 128], F32, tag="ones128")
        nc.vector.memset(ones128[:], 1.0)
        ones_row_r = const.tile([1, 128], F32R, tag="ones_row_r")
        nc.vector.tensor_copy(ones_row_r[:], ones_row_f32[:])
        sel16_s = const.tile([128, BL, BL], F32, tag="sel16_s")
        nc.vector.memset(sel16_s[:], 0.0)
        for b in range(BL):
            nc.vector.memset(sel16_s[:, b, b : b + 1], 1.0)
        sel16 = const.tile([128, BL, BL], F32R, tag="sel16")
        nc.vector.tensor_copy(sel16[:], sel16_s[:])
        ones64_bf = const.tile([64, 64], BF16, tag="ones64_bf")
        nc.vector.tensor_copy(ones64_bf[:], ones64_s[0:64, 0:64])

        g_row_u8 = const.tile([1, B], U8, tag="g_row_u8")
        nc.gpsimd.dma_start(g_row_u8[:], gm_d[None, :])
        g_row = const.tile([1, B], F32, tag="g_row")
        nc.vector.tensor_copy(g_row[:], g_row_u8[:])
        g_col_u8 = const.tile([B, 1], U8, tag="g_col_u8")
        nc.gpsimd.dma_start(g_col_u8[:], gm_d[:, None])
        g_col = const.tile([B, 1], F32, tag="g_col")
        nc.vector.tensor_copy(g_col[:], g_col_u8[:])

        feat_sb = persist.tile([BL, 2 * DIM], F32, tag="feat")
        oo_sb = persist.tile([BL, 2 * DIM], F32, tag="oo")

        # ---------------- o stream: DMA on the sync HWDGE queue (exclusive),
        # sel-matmuls accumulated into a pinned PSUM bank, pumped in small
        # chunks between encoder PE work so the stream never stalls.
        o_pool = stk.enter_context(tc.tile_pool(name="o_pool", bufs=O_BUFS))
        om_ps = ps.tile([BL, ODIM], F32, tag="ps_om", bufs=1, name="ps_om")
        # half-sample granularity: 32 chunks of [128, 4, 512] (1MB each)
        N_OC = 2 * BL
        o_view = o_d.rearrange("b (h n p) d -> (b h) p n d", p=128, h=2)
        o_state = {"issued": 0, "done": 0, "tiles": [None] * N_OC}

        def o_issue():
            c = o_state["issued"]
            if c >= N_OC:
                return
            t = o_pool.tile([128, O_T // 256, ODIM], F32R, tag="o_tile")
            nc.sync.dma_start(t[:], o_view[c])
            o_state["tiles"][c] = t
            o_state["issued"] += 1

        def pump(n=1):
            # n is in chunks (half-samples, ~1MB / ~1.7us of PE work each)
            for _ in range(n):
                c = o_state["done"]
                if c >= N_OC:
                    return
                while o_state["issued"] < min(c + O_BUFS, N_OC):
                    o_issue()
                t = o_state["tiles"][c]
                b = c // 2
                for j in range(O_T // 256):
                    nc.tensor.matmul(
                        om_ps[:],
                        sel16[:, b, :],
                        t[:, j, :],
                        start=(c == 0 and j == 0),
                        stop=(c == N_OC - 1 and j == O_T // 256 - 1),
                    )
                o_state["tiles"][c] = None
                o_state["done"] += 1

        for _ in range(O_BUFS):
            o_issue()

        # ---------------- encoders (pump o matmuls throughout) ----------
        _encoder(nc, tc, pst, persist, const, "rgb", 2048, rgb_d, wd, feat_sb, 0,
                 ident, ones_row_r, ones64_bf, pump)
        _encoder(nc, tc, pst, persist, const, "audio", 128, aud_d, wd,
                 feat_sb, DIM, ident, ones_row_r, ones64_bf, pump)

        # expand weights: loaded late (SBUF freed by the encoders), overlapping
        # the o-stream drain
        expw_pool = stk.enter_context(tc.tile_pool(name="expw", bufs=1))
        expw = expw_pool.tile([128, 4, 2 * DIM], F32R, tag="expw")
        nc.scalar.dma_start(expw[:], wd["expand_W"].rearrange("(c p) d -> p c d", p=128))
        expb = expw_pool.tile([1, 2 * DIM], F32R, tag="expb")
        nc.gpsimd.dma_start(expb[:], wd["expand_b"][None, :])

        pump(2 * BL)  # drain any o chunks not covered by encoder pump points

        if stage == "enc":
            nc.sync.dma_start(dbg_d[0:BL, 0 : 2 * DIM], feat_sb[:])
            return

        # ---------------- o-mean -> expand + normalize -> oo -------------
        om_sb = persist.tile([BL, ODIM], F32, tag="om")
        nc.scalar.activation(om_sb[:], om_ps[:], AF.Copy, scale=1.0 / O_T)
        omT = persist.tile([128, 4, BL], F32R, tag="omT")
        for c in range(4):
            tp = pst([128, BL])
            nc.tensor.transpose(tp[:], om_sb[:, 128 * c : 128 * (c + 1)], ident[:BL, :BL])
            nc.scalar.copy(omT[:, c, :], tp[:])

        oo_ps = []
        for half in range(2):
            pp = pst([BL, DIM])
            for c in range(4):
                nc.tensor.matmul(pp[:], omT[:, c, :], expw[:, c, 512 * half : 512 * (half + 1)],
                                 start=(c == 0), stop=False)
            nc.tensor.matmul(pp[:], ones_row_r[:, :BL], expb[:, 512 * half : 512 * (half + 1)],
                             start=False, stop=True)
            oo_ps.append(pp)
        sq_junk = persist.tile([BL, DIM], F32, tag="sq_junk")
        ss = [persist.tile([BL, 1], F32, tag=f"ss{i}", name=f"ss{i}") for i in range(2)]
        for half in range(2):
            nc.scalar.activation(sq_junk[:], oo_ps[half][:], AF.Square, accum_out=ss[half][:])
        nrm = persist.tile([BL, 1], F32, tag="nrm")
        nc.vector.tensor_tensor(nrm[:], ss[0][:], ss[1][:], AluOpType.add)
        nc.scalar.sqrt(nrm[:], nrm[:])
        nc.vector.tensor_scalar_max(nrm[:], nrm[:], 1e-12)
        rnrm = persist.tile([BL, 1], F32, tag="rnrm")
        nc.vector.reciprocal(rnrm[:], nrm[:])
        for half in range(2):
            nc.vector.tensor_scalar_mul(oo_sb[:, 512 * half : 512 * (half + 1)],
                                        oo_ps[half][:], rnrm[:])

        if stage == "oenc":
            nc.sync.dma_start(dbg_d[0:BL, 0 : 2 * DIM], feat_sb[:])
            nc.sync.dma_start(dbg_d[0:BL, 2 * DIM :], oo_sb[:])
            return

        # ---------------- AllGather (shared-scratchpad output) -----------
        ag_in = dram.tile([BL, 4 * DIM], F32)
        ag_out = dram.tile([B, 4 * DIM], F32, addr_space="Shared")
        nc.scalar.dma_start(ag_in[:, : 2 * DIM], feat_sb[:])
        nc.scalar.dma_start(ag_in[:, 2 * DIM :], oo_sb[:])
        if os.environ.get("KTIME"):
            # collective-free stand-in for TimelineSim (cost model can't model
            # collectives); timing-equivalent except the AllGather.
            nc.scalar.dma_start(ag_out[0:BL, :], ag_in[:])
        else:
            nc.gpsimd.collective_compute(
                "AllGather",
                AluOpType.bypass,
                replica_groups=[list(range(N_CORES))],
                ins=[ag_in.opt()],
                outs=[ag_out.opt()],
            )

        # ---------------- ranking ----------------
        with tc.tile_pool(name="rank", bufs=1) as rank_pool:
            emb = rank_pool.tile([B, 4 * DIM], F32, tag="emb")
            nc.scalar.dma_start(emb[:], ag_out[:])

            if stage == "ag":
                nc.sync.dma_start(dbg_d[:], emb[:])
                return

            # transpose emb -> embT [128, 16, 128]; chunks 0..7 featT, 8..15 ooT
            embT = rank_pool.tile([128, 16, 128], F32, tag="embT")
            for grp4 in range(4):
                tp = pst([128, 512])
                for j in range(4):
                    c = 4 * grp4 + j
                    nc.tensor.transpose(tp[:, 128 * j : 128 * (j + 1)],
                                        emb[:, 128 * c : 128 * (c + 1)], ident[:])
                nc.scalar.copy(embT[:, 4 * grp4 : 4 * grp4 + 4, :],
                               tp[:].rearrange("p (j c) -> p j c", j=4))

            G_ps = pst([B, B])
            for c in range(8):
                nc.tensor.matmul(G_ps[:], embT[:, 8 + c, :], embT[:, c, :],
                                 start=(c == 0), stop=(c == 7))
            G_sb = rank_pool.tile([B, B], F32, tag="G_sb")
            nc.scalar.copy(G_sb[:], G_ps[:])
            # G^T = feat @ oo^T computed directly with swapped operands
            Gt_ps = pst([B, B])
            for c in range(8):
                nc.tensor.matmul(Gt_ps[:], embT[:, c, :], embT[:, 8 + c, :],
                                 start=(c == 0), stop=(c == 7))
            Gt_sb = rank_pool.tile([B, B], F32, tag="Gt_sb")
            nc.scalar.copy(Gt_sb[:], Gt_ps[:])

            if stage == "rank1":
                nc.sync.dma_start(dbg_d[:, 0:B], G_sb[:])
                return

            junk = rank_pool.tile([B, B], F32, tag="junk")
            diag = rank_pool.tile([B, 1], F32, tag="diag")
            nc.vector.tensor_tensor(junk[:], G_sb[:], ident[:], AluOpType.mult)
            nc.vector.reduce_sum(diag[:], junk[:], axis=AX.X)
            mdiag = rank_pool.tile([B, 1], F32, tag="mdiag")
            nc.vector.tensor_scalar(mdiag[:], diag[:], -1.0, MARGIN,
                                    AluOpType.mult, AluOpType.add)

            if stage == "rank1b":
                nc.sync.dma_start(dbg_d[:, 0:B], Gt_sb[:])
                nc.sync.dma_start(dbg_d[:, B : B + 1], diag[:])
                return

            # broadcast g along partitions: gb[m, n] = g[n], via colsums of a
            # zero-padded one-row matrix (K=1 matmuls are avoided).
            g_pad = rank_pool.tile([B, B], F32, tag="g_pad")
            nc.vector.memset(g_pad[:], 0.0)
            nc.vector.tensor_copy(g_pad[0:1, :], g_row[:])
            gb_ps = pst([B, B])
            nc.tensor.matmul(gb_ps[:], ones128[:], g_pad[:], start=True, stop=True)
            gneg_sb = rank_pool.tile([B, B], F32, tag="gneg_sb")
            nc.vector.tensor_scalar(gneg_sb[:], gb_ps[:], 1e30, -1e30,
                                    AluOpType.mult, AluOpType.add)

            stack = rank_pool.tile([B, 6], F32, tag="stack")
            Gm = rank_pool.tile([B, B], F32, tag="Gm")
            rmax = rank_pool.tile([B, 1], F32, tag="rmax")
            top = rank_pool.tile([B, 1], F32, tag="top")
            w = rank_pool.tile([B, 1], F32, tag="w")
            sel = rank_pool.tile([B, 1], F32, tag="sel")
            eq = rank_pool.tile([B, 1], F32, tag="eq")
            colv = rank_pool.tile([B, 1], F32, tag="colv")

            for di, Gsrc in enumerate((G_sb, Gt_sb)):
                T_sb = rank_pool.tile([B, B], F32, tag=f"T{di}")
                nc.scalar.activation(T_sb[:], Gsrc[:], AF.Relu, bias=mdiag[:])
                nc.vector.tensor_tensor(junk[:], T_sb[:], gb_ps[:], AluOpType.mult)
                nc.vector.reduce_sum(w[:], junk[:], axis=AX.X)
                nc.vector.tensor_tensor(stack[:, di : di + 1], w[:], g_col[:], AluOpType.mult)
                nc.vector.tensor_tensor(Gm[:], Gsrc[:], gneg_sb[:], AluOpType.add)
                nc.vector.reduce_max(rmax[:], Gm[:], axis=AX.X)
                nc.vector.tensor_tensor(top[:], diag[:], rmax[:], AluOpType.is_ge)
                nc.vector.tensor_tensor(junk[:], Gsrc[:], gb_ps[:], AluOpType.mult)
                nc.vector.reduce_sum(sel[:], junk[:], axis=AX.X)
                nc.vector.tensor_tensor(sel[:], sel[:], g_col[:], AluOpType.mult)
                nc.vector.tensor_scalar(eq[:], sel[:], 0.0, None, AluOpType.is_equal)
                nc.vector.tensor_scalar(colv[:], eq[:], -1.0, 1.0,
                                        AluOpType.mult, AluOpType.add)
                nc.vector.tensor_copy(stack[:, 4 + di : 5 + di], colv[:])
                nc.vector.tensor_tensor(stack[:, 2 + di : 3 + di], colv[:], top[:],
                                        AluOpType.mult)

            if stage == "rank2":
                nc.sync.dma_start(dbg_d[:, 0:6], stack[:])
                nc.sync.dma_start(dbg_d[:, 8:136], Gt_sb[:])
                return

            S_ps = pst([1, 6])
            nc.tensor.matmul(S_ps[:], ones_col_f32[:], stack[:], start=True, stop=True)
            S_sb = rank_pool.tile([1, 6], F32, tag="S_sb")
            nc.vector.tensor_copy(S_sb[:], S_ps[:])

            if stage == "rank3":
                nc.sync.dma_start(dbg_d[0:1, 0:6], S_sb[:])
                return

            sg = rank_pool.tile([1, 1], F32, tag="sg")
            nc.vector.reduce_sum(sg[:], g_row[:], axis=AX.X)

            def sc(tag):
                return rank_pool.tile([1, 1], F32, tag=tag, name=tag)

            t_ls = sc("t_ls")
            nc.vector.tensor_tensor(t_ls[:], S_sb[:, 0:1], S_sb[:, 1:2], AluOpType.add)
            num = sc("num")
            nc.vector.tensor_scalar_mul(num[:], sg[:], -2.0 * MARGIN)
            nc.vector.tensor_tensor(num[:], num[:], t_ls[:], AluOpType.add)
            d1 = sc("d1")
            nc.vector.tensor_scalar(d1[:], sg[:], -1.0, 1.0, AluOpType.add, AluOpType.max)
            ind = sc("ind")
            nc.vector.tensor_scalar(ind[:], sg[:], -1.0, 0.0, AluOpType.add, AluOpType.max)
            nc.vector.tensor_scalar_min(ind[:], ind[:], 1.0)
            nv = sc("nv")
            nc.vector.tensor_tensor(nv[:], ind[:], sg[:], AluOpType.mult)
            d2 = sc("d2")
            nc.vector.tensor_scalar_max(d2[:], nv[:], 1.0)
            r1 = sc("r1")
            nc.vector.reciprocal(r1[:], d1[:])
            r2 = sc("r2")
            nc.vector.reciprocal(r2[:], d2[:])
            out_sb = rank_pool.tile([1, 2], F32, tag="out_sb")
            nc.vector.tensor_tensor(num[:], num[:], r1[:], AluOpType.mult)
            nc.vector.tensor_tensor(out_sb[:, 0:1], num[:], r2[:], AluOpType.mult)

            acc_h = []
            for di in range(2):
                nvx = sc(f"nvx{di}")
                nc.vector.tensor_scalar_max(nvx[:], S_sb[:, 4 + di : 5 + di], 1.0)
                rx = sc(f"rx{di}")
                nc.vector.reciprocal(rx[:], nvx[:])
                ax = sc(f"ax{di}")
                nc.vector.tensor_tensor(ax[:], S_sb[:, 2 + di : 3 + di], rx[:], AluOpType.mult)
                acc_h.append(ax)
            asum = sc("asum")
            nc.vector.tensor_tensor(asum[:], acc_h[0][:], acc_h[1][:], AluOpType.add)
            nc.vector.tensor_scalar_mul(out_sb[:, 1:2], asum[:], 0.5)

            nc.sync.dma_start(out_d[:], out_sb[:])


def _encoder(nc, tc, pst, persist, const, mod, dm, x_d, wd, feat_sb, feat_off,
             ident, ones_row_r, ones64_bf, pump):
    """Self-attention pooled encoder; writes feat_sb[:, feat_off:feat_off+512].

    All DMAs go on the scalar HWDGE / gpsimd queues (the sync queue is
    reserved for the o stream). `pump()` is called between PE chunks to
    interleave the o-mean matmuls.
    """
    K = dm // 128
    n_tt = TOK // 128  # 8

    enc_pool_cm = tc.tile_pool(name=f"enc_{mod}", bufs=1)
    enc = enc_pool_cm.__enter__()
    qT = enc.tile([128, HEADS, TOK], BF16, tag="qT")
    kT = enc.tile([128, HEADS, TOK], BF16, tag="kT")
    v_sb = enc.tile([128, n_tt, DIM], BF16, tag="v_sb")
    poolT = enc.tile([128, HEADS, BL], F32R, tag="poolT")

    with ExitStack() as estk:
        xT_pool = estk.enter_context(tc.tile_pool(name=f"xT_{mod}", bufs=1))
        xT = xT_pool.tile([128, K, TOK], F32R, tag="xT")
        flat = x_d.rearrange("b t d -> (b t) d")
        wv_cm = tc.tile_pool(name=f"wv_{mod}", bufs=1)
        wv_pool = wv_cm.__enter__()
        wv = wv_pool.tile([128, K, DIM], F32R, tag="wv")
        bv = wv_pool.tile([1, DIM], F32R, tag="bv")
        with tc.tile_pool(name=f"xload_{mod}", bufs=2) as xload:
            if dm == 128:
                x_nat = xload.tile([128, n_tt, 128], F32, tag="x_nat_a")
                nc.scalar.dma_start(x_nat[:], flat.rearrange("(n p) d -> p n d", p=128))
                nc.scalar.dma_start(wv[:], wd[f"{mod}_Wv"].rearrange("(kc p) d -> p kc d", p=128))
                nc.gpsimd.dma_start(bv[:], wd[f"{mod}_bv"][None, :])
                for tt in range(n_tt):
                    tp = pst([128, 512])
                    nc.tensor.transpose(tp[:, :128], x_nat[:, tt, :], ident[:])
                    nc.scalar.copy(xT[:, 0, 128 * tt : 128 * (tt + 1)], tp[:, :128])
            else:
                # stage the first x tile before the big Wv load so PE can
                # start transposing at ~3us
                x0 = xload.tile([128, dm], F32, tag="x_nat", name="x_nat")
                x_nats = [x0]
                nc.scalar.dma_start(x_nats[0][:], flat[0:128, :])
                nc.scalar.dma_start(wv[:], wd[f"{mod}_Wv"].rearrange("(kc p) d -> p kc d", p=128))
                nc.gpsimd.dma_start(bv[:], wd[f"{mod}_bv"][None, :])
                for tt in range(n_tt):
                    if tt + 1 < n_tt:
                        nxt = xload.tile([128, dm], F32, tag="x_nat", name="x_nat")
                        nc.scalar.dma_start(nxt[:], flat[128 * (tt + 1) : 128 * (tt + 2), :])
                        x_nats.append(nxt)
                    x_nat = x_nats[tt]
                    for kc4 in range(K // 4):
                        tp = pst([128, 512])
                        for j in range(4):
                            kc = 4 * kc4 + j
                            nc.tensor.transpose(tp[:, 128 * j : 128 * (j + 1)],
                                                x_nat[:, 128 * kc : 128 * (kc + 1)], ident[:])
                        nc.scalar.copy(xT[:, 4 * kc4 : 4 * kc4 + 4, 128 * tt : 128 * (tt + 1)],
                                       tp[:].rearrange("p (j c) -> p j c", j=4))
                    if tt in (3, 7):
                        pump(1)

        # v: lhsT = xT token-tile (stationary), rhs = Wv k-rows (moving)
        for tt in range(n_tt):
            pv = pst([128, DIM])
            for kc in range(K):
                nc.tensor.matmul(pv[:], xT[:, kc, 128 * tt : 128 * (tt + 1)], wv[:, kc, :],
                                 start=(kc == 0), stop=False)
            nc.tensor.matmul(pv[:], ones_row_r[:], bv[:], start=False, stop=True)
            nc.vector.tensor_copy(v_sb[:, tt, :], pv[:])
            if tt > 0:
                pump(1)
        wv_cm.__exit__(None, None, None)

        # q, k: lhsT = W column-block (stationary), rhs = xT (moving) -> [d, tok]
        bq_sb = const.tile([128, HEADS], F32, tag=f"bq_{mod}")
        nc.gpsimd.dma_start(bq_sb[:], wd[f"{mod}_bq"].rearrange("(o p) -> p o", p=128))
        bk_sb = const.tile([128, HEADS], F32, tag=f"bk_{mod}")
        nc.gpsimd.dma_start(bk_sb[:], wd[f"{mod}_bk"].rearrange("(o p) -> p o", p=128))
        with tc.tile_pool(name=f"wcol_{mod}", bufs=2) as wcol_pool:
            for pname, outT, b_sb in (("q", qT, bq_sb), ("k", kT, bk_sb)):
                w_d = wd[f"{mod}_W{pname}"].rearrange("(kc p) d -> p kc d", p=128)
                if K == 1:
                    wfull = wcol_pool.tile([128, DIM], F32R, tag="wfull", name="wfull")
                    nc.scalar.dma_start(wfull[:], w_d[:, 0, :])
                for dt_ in range(HEADS):
                    if K == 1:
                        wcol = wfull[:, None, 128 * dt_ : 128 * (dt_ + 1)]
                    else:
                        wcol = wcol_pool.tile([128, K, 128], F32R, tag="wcol",
                                              name="wcol")
                        nc.scalar.dma_start(wcol[:],
                                            w_d[:, :, 128 * dt_ : 128 * (dt_ + 1)])
                    for blk in range(TOK // 512):
                        pq = pst([128, 512])
                        for kc in range(K):
                            nc.tensor.matmul(pq[:], wcol[:, kc, :],
                                             xT[:, kc, 512 * blk : 512 * (blk + 1)],
                                             start=(kc == 0), stop=(kc == K - 1))
                        nc.scalar.activation(outT[:, dt_, 512 * blk : 512 * (blk + 1)], pq[:],
                                             AF.Identity, bias=b_sb[:, dt_ : dt_ + 1])
                    pump(2)

    # attention, grp-outer: reshuffle 8 samples of v to base partition 0 via
    # SBUF->SBUF DMA (engines cannot shift partitions), then per-head flow.
    scale = 1.0 / math.sqrt(HD)
    with ExitStack() as lstk:
        late = lstk.enter_context(tc.tile_pool(name=f"late_{mod}", bufs=1))
        avT = late.tile([128, HEADS, TOK], F32R, tag="avT")
        wo_pool = lstk.enter_context(tc.tile_pool(name=f"wo_{mod}", bufs=1))
        wo = wo_pool.tile([128, HEADS, DIM], F32R, tag="wo")
        nc.scalar.dma_start(wo[:], wd[f"{mod}_Wo"].rearrange("(h p) d -> p h d", p=128))
        w2 = wo_pool.tile([128, HEADS, DIM], F32R, tag="w2")
        nc.scalar.dma_start(w2[:], wd[f"{mod}_W2"].rearrange("(c p) d -> p c d", p=128))
        b2 = wo_pool.tile([1, DIM], F32R, tag="b2")
        nc.gpsimd.dma_start(b2[:], wd[f"{mod}_b2"][None, :])
        bo_sb = const.tile([128, HEADS], F32, tag=f"bo_{mod}")
        nc.gpsimd.dma_start(bo_sb[:], wd[f"{mod}_bo"].rearrange("(o p) -> p o", p=128))
        ap = lstk.enter_context(tc.tile_pool(name=f"attn_{mod}", bufs=3))
        for grp in range(BL // 8):
            v8 = ap.tile([64, 8, DIM], BF16, tag="v8")
            v8v = v8[:].rearrange("p (ul half) d -> p ul half d", half=2)
            nc.scalar.dma_start(v8v[:, :, 0, :], v_sb[0:64, 4 * grp : 4 * grp + 4, :])
            nc.scalar.dma_start(v8v[:, :, 1, :], v_sb[64:128, 4 * grp : 4 * grp + 4, :])
            for h in range(HEADS):
                sT8 = pst([64, 512])
                for i in range(8):
                    b = 8 * grp + i
                    nc.tensor.matmul(sT8[:, 64 * i : 64 * (i + 1)],
                                     kT[:, h, 64 * b : 64 * (b + 1)],
                                     qT[:, h, 64 * b : 64 * (b + 1)],
                                     start=True, stop=True)
                exps = ap.tile([64, 512], BF16, tag="exps")
                nc.scalar.activation(exps[:], sT8[:], AF.Exp, scale=scale)
                rs = pst([64, 512])
                nc.tensor.matmul(rs[:], ones64_bf[:], exps[:],
                                 start=True, stop=True)
                rrs = ap.tile([64, 512], F32, tag="rrs")
                nc.vector.reciprocal(rrs[:], rs[:])
                aT8 = ap.tile([64, 512], BF16, tag="aT8")
                nc.vector.tensor_tensor(aT8[:], exps[:], rrs[:], AluOpType.mult)
                avp = pst([128, 512])
                for i in range(8):
                    nc.tensor.matmul(avp[:, 64 * i : 64 * (i + 1)],
                                     v8[:, i, 128 * h : 128 * (h + 1)],
                                     aT8[:, 64 * i : 64 * (i + 1)],
                                     start=True, stop=True)
                nc.vector.tensor_copy(avT[:, h, 512 * grp : 512 * (grp + 1)], avp[:])
            pump(2)

        # out-proj (transposed) + time pooling + W2
        red = wo_pool.tile([128, 8], F32, tag="red")
        for dt_ in range(HEADS):
            for blk in range(TOK // 512):
                pp = pst([128, 512])
                for h in range(HEADS):
                    nc.tensor.matmul(pp[:], wo[:, h, 128 * dt_ : 128 * (dt_ + 1)],
                                     avT[:, h, 512 * blk : 512 * (blk + 1)],
                                     start=(h == 0), stop=(h == HEADS - 1))
                nc.vector.reduce_sum(red[:], pp[:].rearrange("p (s t) -> p s t", t=T),
                                     axis=AX.X)
                nc.vector.tensor_scalar(poolT[:, dt_, 8 * blk : 8 * blk + 8], red[:],
                                        1.0 / T, bo_sb[:, dt_ : dt_ + 1],
                                        AluOpType.mult, AluOpType.add)
            pump(1)

        pf = pst([BL, DIM])
        for c in range(HEADS):
            nc.tensor.matmul(pf[:], poolT[:, c, :], w2[:, c, :], start=(c == 0), stop=False)
        nc.tensor.matmul(pf[:], ones_row_r[:, :BL], b2[:], start=False, stop=True)
        nc.scalar.copy(feat_sb[:, feat_off : feat_off + DIM], pf[:])

    enc_pool_cm.__exit__(None, None, None)


def kernel(**inputs):
    if "runner" not in _CACHE:
        _CACHE["runner"] = _make_runner()
    return _CACHE["runner"](inputs)


def _make_runner():
    nc = _build()
    import jax
    from jax.sharding import Mesh, PartitionSpec
    from jax.experimental.shard_map import shard_map
    from concourse import bass2jax

    bass2jax.install_neuronx_cc_hook()

    partition_name = nc.partition_id_tensor.name if nc.partition_id_tensor else None
    in_names, out_names, out_avals, zero_outs = [], [], [], []
    for alloc in nc.m.functions[0].allocations:
        if not isinstance(alloc, mybir.MemoryLocationSet):
            continue
        name = alloc.memorylocations[0].name
        if alloc.kind == "ExternalInput":
            if name != partition_name:
                in_names.append(name)
        elif alloc.kind == "ExternalOutput":
            out_names.append(name)
            shape = tuple(alloc.tensor_shape)
            dtype = mybir.dt.np(alloc.dtype)
            out_avals.append(jax.core.ShapedArray(shape, dtype))
            zero_outs.append(np.zeros(shape, dtype))
    n_params = len(in_names)
    all_in_names = list(in_names) + list(out_names)
    if partition_name is not None:
        all_in_names.append(partition_name)

    def _body(*args):
        operands = list(args)
        if partition_name is not None:
            operands.append(bass2jax.partition_id_tensor())
        outs = bass2jax._bass_exec_p.bind(
            *operands,
            out_avals=tuple(out_avals),
            in_names=tuple(all_in_names),
            out_names=tuple(out_names),
            lowering_input_output_aliases=(),
            sim_require_finite=True,
            sim_require_nnan=True,
            nc=nc,
        )
        return tuple(outs)

    devices = jax.devices()[:N_CORES]
    mesh = Mesh(np.asarray(devices), ("core",))
    in_specs = (PartitionSpec("core"),) * (n_params + len(out_names))
    out_specs = (PartitionSpec("core"),) * len(out_names)
    sharded = jax.jit(
        shard_map(_body, mesh=mesh, in_specs=in_specs, out_specs=out_specs,
                  check_rep=False),
        keep_unused=True,
    )

    out_idx = out_names.index("out")

    def run(inputs):
        per_core = _shard_inputs(inputs)
        concat_in = [
            np.concatenate([per_core[c][name] for c in range(N_CORES)], axis=0)
            for name in in_names
        ]
        concat_zeros = [
            np.zeros((N_CORES * z.shape[0], *z.shape[1:]), z.dtype) for z in zero_outs
        ]
        out_arrs = sharded(*concat_in, *concat_zeros)
        run.last_outputs = {n: np.asarray(out_arrs[i]) for i, n in enumerate(out_names)}
        out = run.last_outputs["out"]  # [8, 2]
        return np.float32(out[0, 0]), np.float32(out[0, 1])

    run.sharded = sharded
    run.in_names = in_names
    run.zero_outs = zero_outs
    run.nc = nc
    return run


def _shard_inputs(inputs):
    per_core = []
    gm = np.ascontiguousarray(np.asarray(inputs["group_mask"]).astype(np.uint8))
    shared = {}
    for k, v in inputs.items():
        if k not in ("o", "rgb", "audio", "group_mask"):
            shared[k] = np.ascontiguousarray(np.asarray(v, dtype=np.float32))
    o = np.asarray(inputs["o"], dtype=np.float32)
    rgb = np.asarray(inputs["rgb"], dtype=np.float32)
    audio = np.asarray(inputs["audio"], dtype=np.float32)
    for c in range(N_CORES):
        sl = slice(BL * c, BL * (c + 1))
        m = {
            "o": np.ascontiguousarray(o[sl]),
            "rgb": np.ascontiguousarray(rgb[sl]),
            "audio": np.ascontiguousarray(audio[sl]),
            "group_mask": gm,
        }
        m.update(shared)
        per_core.append(m)
    return per_core
